# revision 9
# baseline (speedup 1.0000x reference)
"""BinaryTreeGRU Trainium2 kernel.

Batch of B=64 complete binary trees (L=512 leaves, 1023 nodes each),
data-parallel over trees across 8 NeuronCores (8 trees/core).

Layout: feature-major ("folded"): every activation tensor lives in SBUF as
[128 partitions, 2 feature-blocks, n_nodes] (mem dim 256 = 2 blocks of 128).
Level l has N_l = 8 * 512 / 2^l node-columns per core, columns packed
tree-major so that node j's children sit at columns 2j and 2j+1 of the
previous level -- child selection is a stride-2 access pattern, no copies.

Per level:  rzh = Wrzh @ [h_l ; h_r]  (PE, fp32r, N=512 moving chunks)
            gates = sigmoid(rzh + b)  (ACT, fused across gate blocks)
            s = r_l*h_l + r_r*h_r     (GpSimd + DVE)
            g = tanh(Wgh @ s + bg)    (PE + ACT)
            h = z_l*(h_l - g/2) + z_r*(h_r - g/2) + g   (DVE/GpSimd)

Host side only reshapes/transposes numpy arrays for sharding and gathers
the result.
"""

import os
from contextlib import ExitStack

import numpy as np

import concourse.bass as bass
import concourse.mybir as mybir
import concourse.tile as tile
from concourse import bacc
from concourse.bass_utils import run_bass_kernel_spmd

F32 = mybir.dt.float32
F32R = mybir.dt.float32r
MULT = mybir.AluOpType.mult
ADD = mybir.AluOpType.add
SIGMOID = mybir.ActivationFunctionType.Sigmoid
TANH = mybir.ActivationFunctionType.Tanh

MEM = 256
IN_DIM = 256
B = 64
L = 512
NCORES = 8
BLOC = B // NCORES            # trees per core
N0 = BLOC * L                 # leaf columns per core = 4096
NLEVELS = 10                  # 4096,2048,...,8 columns
NCOLS = [N0 >> l for l in range(NLEVELS)]
TOT = sum(NCOLS)              # 8184
OFFS = np.cumsum([0] + NCOLS).tolist()
NC = 512                      # node-column chunk (fp32 matmul moving max)

# W-row 128-blocks of Wrzh arranged column order (quarters of the big
# gates tile):
#   q0=[r_l0, z_l0]  q1=[r_l1, z_l1]  q2=[r_r0, z_r0]  q3=[r_r1, z_r1]
# Wrzh rows: r_l=0:256, r_r=256:512, z_l=512:768, z_r=768:1024
WRZH_PERM = [0, 4, 1, 5, 2, 6, 3, 7]

LAST_RESULT = {}


def _wavefront_order(nchunks, d=2):
    """Topological chunk order interleaving levels.

    Chunk (lv, ci)'s parents are (lv-1, 2ci) and (lv-1, 2ci+1) when level
    lv-1 has 2x the chunks; when levels shrink below NC the parent is the
    single previous-level chunk. Child front must come >= parent_pos + d
    (parent back emitted d steps after its front). Returns list of (lv, ci).
    """
    pos = {}
    order = []
    remaining = [(lv, ci) for lv in range(NLEVELS)
                 for ci in range(nchunks[lv])]

    def parents(lv, ci):
        if lv == 0:
            return []
        if nchunks[lv - 1] == 2 * nchunks[lv]:
            return [(lv - 1, 2 * ci), (lv - 1, 2 * ci + 1)]
        return [(lv - 1, pc) for pc in range(nchunks[lv - 1])]

    t = 0
    while remaining:
        ready = []
        for (lv, ci) in remaining:
            ps = parents(lv, ci)
            dd = d if lv > 1 else (d if lv == 1 else 0)
            if all(p in pos and pos[p] + (1 if lv == 1 else dd) <= t
                   for p in ps):
                ready.append((lv, ci))
        if ready:
            ch = max(ready, key=lambda c: (c[0], -c[1]))
            pos[ch] = t
            order.append(ch)
            remaining.remove(ch)
        else:
            order.append(None)   # spacing step (emit only a back)
        t += 1
    return order


def build_nc(fast_bias: bool):
    nc = bacc.Bacc("TRN2", target_bir_lowering=False, debug=False)

    d_x = nc.dram_tensor("xT", [2, 128, N0], F32R, kind="ExternalInput")
    d_wrzh = nc.dram_tensor("wrzh", [4, 128, 1024], F32R, kind="ExternalInput")
    d_wgrzx = nc.dram_tensor("wgrzx", [2, 128, 768], F32R, kind="ExternalInput")
    d_wgh = nc.dram_tensor("wgh", [2, 128, 256], F32R, kind="ExternalInput")
    d_bias = nc.dram_tensor("bias6", [6, 128, 1], F32, kind="ExternalInput")
    d_out = nc.dram_tensor("out", [2, 128, TOT], F32R, kind="ExternalOutput")

    x = d_x.ap()
    wrzh = d_wrzh.ap()
    wgrzx = d_wgrzx.ap()
    wgh = d_wgh.ap()
    bias6 = d_bias.ap()
    out = d_out.ap()

    mm = nc.tensor.matmul
    nchunks = [max(1, NCOLS[lv] // NC) for lv in range(NLEVELS)]

    with tile.TileContext(nc) as tc, ExitStack() as ctx:
        singles = ctx.enter_context(tc.tile_pool(name="singles", bufs=1))
        xpool = ctx.enter_context(tc.tile_pool(name="xpool", bufs=3))
        gates_pool = ctx.enter_context(tc.tile_pool(name="gates", bufs=3))
        gsb_pool = ctx.enter_context(tc.tile_pool(name="gsb", bufs=3))
        scratch = ctx.enter_context(tc.tile_pool(name="scratch", bufs=2))
        psum = ctx.enter_context(tc.tile_pool(name="psum", bufs=4, space="PSUM"))

        # --- load constants ---
        w_rzh = []
        for kc in range(4):
            t = singles.tile([128, 1024], F32R, tag=f"wrzh{kc}", name=f"wrzh{kc}")
            nc.sync.dma_start(out=t, in_=wrzh[kc])
            w_rzh.append(t)
        w_grzx = []
        for kc in range(2):
            t = singles.tile([128, 768], F32R, tag=f"wgrzx{kc}", name=f"wgrzx{kc}")
            nc.sync.dma_start(out=t, in_=wgrzx[kc])
            w_grzx.append(t)
        w_gh = []
        for kc in range(2):
            t = singles.tile([128, 256], F32R, tag=f"wgh{kc}", name=f"wgh{kc}")
            nc.sync.dma_start(out=t, in_=wgh[kc])
            w_gh.append(t)
        b_t = []
        for i in range(6):
            t = singles.tile([128, 1], F32, tag=f"b{i}", name=f"b{i}")
            nc.sync.dma_start(out=t, in_=bias6[i])
            b_t.append(t)
        # b_t: [0]=bg0 [1]=bg1 [2]=bA0 [3]=bA1 [4]=bB0 [5]=bB1

        h_t = [singles.tile([128, 2, NCOLS[l]], F32R, tag=f"h{l}",
                            name=f"h{l}")
               for l in range(NLEVELS)]

        state = {}   # (lv, ci) -> dict of tiles/views for the back phase

        def emit_leaf_front(ci):
            c0 = ci * NC
            x_c = []
            for kc in range(2):
                t = xpool.tile([128, NC], F32R, tag=f"x{kc}", name=f"x{kc}")
                nc.sync.dma_start(out=t, in_=x[kc, :, c0:c0 + NC])
                x_c.append(t)
            srzx = gates_pool.tile([128, 4, NC], F32, tag="gates", name="srzx")
            for q in range(2):
                ps = psum.tile([128, 2, NC], F32, tag="ps", name="ps_rzx")
                for mb in range(2):
                    col = 256 + (q * 2 + mb) * 128
                    for kc in range(2):
                        mm(ps[:, mb, :], w_grzx[kc][:, col:col + 128],
                           x_c[kc], start=(kc == 0), stop=(kc == 1))
                if fast_bias:
                    nc.scalar.activation(srzx[:, 2 * q:2 * q + 2, :], ps,
                                         SIGMOID, bias=1.0)
                else:
                    for mb in range(2):
                        nc.scalar.activation(
                            srzx[:, 2 * q + mb, :], ps[:, mb, :],
                            SIGMOID, bias=b_t[2 + 2 * q + mb])
            ps_gx = psum.tile([128, 2, NC], F32, tag="ps", name="ps_gx")
            for mb in range(2):
                for kc in range(2):
                    mm(ps_gx[:, mb, :], w_grzx[kc][:, 128 * mb:128 * mb + 128],
                       x_c[kc], start=(kc == 0), stop=(kc == 1))
            tg = gsb_pool.tile([128, 2, NC], F32, tag="gsb", name="tg")
            if fast_bias:
                nc.scalar.activation(tg, ps_gx, TANH, bias=0.0)
            else:
                for mb in range(2):
                    nc.scalar.activation(tg[:, mb, :], ps_gx[:, mb, :],
                                         TANH, bias=b_t[mb])
            zsum = scratch.tile([128, 2, NC], F32, tag="sa", name="zsum")
            nc.gpsimd.tensor_add(zsum, srzx[:, 0:2, :], srzx[:, 2:4, :])
            tt = scratch.tile([128, 2, NC], F32, tag="sb", name="tt")
            nc.vector.tensor_scalar(tt, zsum, -0.5, 1.0, MULT, ADD)
            nc.vector.tensor_mul(h_t[0][:, :, c0:c0 + NC], tt, tg)
            if ci == nchunks[0] - 1:
                for cb in range(2):
                    nc.sync.dma_start(out=out[cb, :, OFFS[0]:OFFS[1]],
                                      in_=h_t[0][:, cb, :])

        def emit_front(lv, ci):
            if lv == 0:
                emit_leaf_front(ci)
                return
            n = NCOLS[lv]
            hp = h_t[lv - 1]
            ncur = min(n, NC)
            c0 = ci * ncur
            ch = hp[:, :, 2 * c0:2 * c0 + 2 * ncur].rearrange(
                "p c (n two) -> p c two n", two=2)
            h_l = ch[:, :, 0, :]
            h_r = ch[:, :, 1, :]

            # gates blocks: [rl0, zl0, rl1, zl1, rr0, zr0, rr1, zr1]
            gt = gates_pool.tile([128, 8, ncur], F32, tag="gates", name="gt")
            for q in range(4):
                ps = psum.tile([128, 2, ncur], F32, tag="ps", name="ps_rz")
                for mb in range(2):
                    col = (q * 2 + mb) * 128
                    for kc in range(4):
                        mm(ps[:, mb, :], w_rzh[kc][:, col:col + 128],
                           ch[:, kc % 2, kc // 2, :],
                           start=(kc == 0), stop=(kc == 3))
                if fast_bias:
                    nc.scalar.activation(gt[:, 2 * q:2 * q + 2, :], ps,
                                         SIGMOID, bias=1.0)
                else:
                    bi = (2, 3, 4, 5)[q]
                    nc.scalar.activation(gt[:, 2 * q:2 * q + 2, :], ps,
                                         SIGMOID, bias=b_t[bi])
            glv = gt[:, 0:4, :].rearrange("p (c two) n -> p two c n", two=2)
            grv = gt[:, 4:8, :].rearrange("p (c two) n -> p two c n", two=2)
            r_l, z_l = glv[:, 0], glv[:, 1]
            r_r, z_r = grv[:, 0], grv[:, 1]

            big = ncur >= 256
            e1 = nc.gpsimd if big else nc.vector
            p = scratch.tile([128, 2, ncur], F32, tag="sa", name="p")
            q_ = scratch.tile([128, 2, ncur], F32, tag="sb", name="q")
            s = scratch.tile([128, 2, ncur], F32R, tag="sc", name="s")
            e1.tensor_mul(p, r_l, h_l)
            e1.tensor_mul(q_, r_r, h_r)
            nc.vector.tensor_add(s, p, q_)
            state[(lv, ci)] = dict(s=s, z_l=z_l, z_r=z_r, h_l=h_l, h_r=h_r,
                                   c0=c0, ncur=ncur, big=big)

        def emit_back(lv, ci):
            if lv == 0:
                return
            st = state.pop((lv, ci))
            s, z_l, z_r = st["s"], st["z_l"], st["z_r"]
            h_l, h_r = st["h_l"], st["h_r"]
            c0, ncur, big = st["c0"], st["ncur"], st["big"]

            psg = psum.tile([128, 2, ncur], F32, tag="ps", name="ps_g")
            for mb in range(2):
                for kc in range(2):
                    mm(psg[:, mb, :], w_gh[kc][:, 128 * mb:128 * mb + 128],
                       s[:, kc, :], start=(kc == 0), stop=(kc == 1))
            g_sb = gsb_pool.tile([128, 2, ncur], F32, tag="gsb", name="g_sb")
            if fast_bias:
                nc.scalar.activation(g_sb, psg, TANH, bias=0.0)
            else:
                for mb in range(2):
                    nc.scalar.activation(g_sb[:, mb, :], psg[:, mb, :],
                                         TANH, bias=b_t[mb])

            e2 = nc.gpsimd if big else nc.vector
            u_l = scratch.tile([128, 2, ncur], F32, tag="sd", name="u_l")
            u_r = scratch.tile([128, 2, ncur], F32, tag="se", name="u_r")
            nc.vector.scalar_tensor_tensor(u_l, g_sb, -0.5, h_l, MULT, ADD)
            nc.vector.scalar_tensor_tensor(u_r, g_sb, -0.5, h_r, MULT, ADD)
            v = scratch.tile([128, 2, ncur], F32, tag="sa", name="v")
            w = scratch.tile([128, 2, ncur], F32, tag="sb", name="w")
            e2.tensor_mul(v, z_l, u_l)
            e2.tensor_mul(w, z_r, u_r)
            xx = scratch.tile([128, 2, ncur], F32, tag="sd", name="xx")
            nc.vector.tensor_add(xx, v, w)
            nc.vector.tensor_add(h_t[lv][:, :, c0:c0 + ncur], xx, g_sb)
            if ci == nchunks[lv] - 1:
                for cb in range(2):
                    nc.sync.dma_start(out=out[cb, :, OFFS[lv]:OFFS[lv + 1]],
                                      in_=h_t[lv][:, cb, :])

        D = 2
        order = _wavefront_order(nchunks, D)

        def parent_list(lv, ci):
            if lv == 0:
                return []
            if nchunks[lv - 1] == 2 * nchunks[lv]:
                return [(lv - 1, 2 * ci), (lv - 1, 2 * ci + 1)]
            return [(lv - 1, pc) for pc in range(nchunks[lv - 1])]

        pending = []
        done = set()

        def pop_back():
            b = pending.pop(0)
            emit_back(*b)
            done.add(b)

        for ch in order:
            if ch is None:
                if pending:
                    pop_back()
                continue
            lv, ci = ch
            for par in parent_list(lv, ci):
                while par not in done:
                    pop_back()
            while len(pending) >= D:
                pop_back()
            emit_front(lv, ci)
            pending.append(ch)
        while pending:
            pop_back()

    nc.compile()
    return nc


def _prep_inputs(inputs, Wgrzx, bgrzx, Wrzh, Wgh):
    """Host-side shard + layout prep. Returns (in_maps, fast_bias)."""
    x = np.ascontiguousarray(inputs, dtype=np.float32)
    Wgrzx = np.asarray(Wgrzx, dtype=np.float32)
    bgrzx = np.asarray(bgrzx, dtype=np.float32)
    Wrzh = np.asarray(Wrzh, dtype=np.float32)
    Wgh = np.asarray(Wgh, dtype=np.float32)

    fast_bias = bool(
        np.all(bgrzx[:MEM] == 0.0) and np.all(bgrzx[MEM:] == 1.0))

    wgrzxT = np.ascontiguousarray(Wgrzx.T.reshape(2, 128, 768))
    wrzhT = Wrzh.T.reshape(512, 8, 128)
    wrzhT = np.ascontiguousarray(
        wrzhT[:, WRZH_PERM, :].reshape(512, 1024).reshape(4, 128, 1024))
    wghT = np.ascontiguousarray(Wgh.T.reshape(2, 128, 256))
    bias6 = np.ascontiguousarray(bgrzx.reshape(6, 128, 1))

    in_maps = []
    for c in range(NCORES):
        xc = x[c * BLOC:(c + 1) * BLOC].reshape(N0, IN_DIM)
        xT = np.ascontiguousarray(xc.T).reshape(2, 128, N0)
        in_maps.append({
            "xT": xT,
            "wrzh": wrzhT,
            "wgrzx": wgrzxT,
            "wgh": wghT,
            "bias6": bias6,
        })
    return in_maps, fast_bias


def _gather(results):
    """results: list of per-core {'out': [2,128,TOT]} -> [B, 2L-1, MEM]."""
    outs = []
    for c in range(len(results)):
        fm = results[c]["out"].reshape(MEM, TOT)
        levels = []
        for lv in range(NLEVELS):
            k = NCOLS[lv] // BLOC
            blk = fm[:, OFFS[lv]:OFFS[lv + 1]].reshape(MEM, BLOC, k)
            levels.append(blk.transpose(1, 2, 0))
        outs.append(np.concatenate(levels, axis=1))
    return np.ascontiguousarray(
        np.concatenate(outs, axis=0), dtype=np.float32)


def kernel(**inputs):
    in_maps, fast_bias = _prep_inputs(
        inputs["inputs"], inputs["Wgrzx"], inputs["bgrzx"],
        inputs["Wrzh"], inputs["Wgh"])
    nc = build_nc(fast_bias)
    trace = bool(int(os.environ.get("BTGRU_TRACE", "0")))
    res = run_bass_kernel_spmd(
        nc, in_maps, core_ids=list(range(NCORES)), trace=trace)
    LAST_RESULT.clear()
    LAST_RESULT["exec_time_ns"] = res.exec_time_ns
    LAST_RESULT["profile_json"] = res.profile_json
    return _gather(res.results)


# revision 11
# speedup vs baseline: 1.0650x; 1.0650x over previous
"""BinaryTreeGRU Trainium2 kernel.

Batch of B=64 complete binary trees (L=512 leaves, 1023 nodes each),
data-parallel over trees across 8 NeuronCores (8 trees/core).

Layout: feature-major ("folded"): every activation tensor lives in SBUF as
[128 partitions, 2 feature-blocks, n_nodes] (mem dim 256 = 2 blocks of 128).
Level l has N_l = 8 * 512 / 2^l node-columns per core, columns packed
tree-major so that node j's children sit at columns 2j and 2j+1 of the
previous level -- child selection is a stride-2 access pattern, no copies.

Per level:  rzh = Wrzh @ [h_l ; h_r]  (PE, fp32r, N=512 moving chunks)
            gates = sigmoid(rzh + b)  (ACT, fused across gate blocks)
            s = r_l*h_l + r_r*h_r     (GpSimd + DVE)
            g = tanh(Wgh @ s + bg)    (PE + ACT)
            h = z_l*(h_l - g/2) + z_r*(h_r - g/2) + g   (DVE/GpSimd)

Host side only reshapes/transposes numpy arrays for sharding and gathers
the result.
"""

import os
from contextlib import ExitStack

import numpy as np

import concourse.bass as bass
import concourse.mybir as mybir
import concourse.tile as tile
from concourse import bacc
from concourse.bass_utils import run_bass_kernel_spmd

F32 = mybir.dt.float32
F32R = mybir.dt.float32r
MULT = mybir.AluOpType.mult
ADD = mybir.AluOpType.add
SIGMOID = mybir.ActivationFunctionType.Sigmoid
TANH = mybir.ActivationFunctionType.Tanh

MEM = 256
IN_DIM = 256
B = 64
L = 512
NCORES = 8
BLOC = B // NCORES            # trees per core
N0 = BLOC * L                 # leaf columns per core = 4096
NLEVELS = 10                  # 4096,2048,...,8 columns
NCOLS = [N0 >> l for l in range(NLEVELS)]
TOT = sum(NCOLS)              # 8184
OFFS = np.cumsum([0] + NCOLS).tolist()
NC = 512                      # node-column chunk (fp32 matmul moving max)

# W-row 128-blocks of Wrzh arranged column order (quarters of the big
# gates tile):
#   q0=[r_l0, z_l0]  q1=[r_l1, z_l1]  q2=[r_r0, z_r0]  q3=[r_r1, z_r1]
# Wrzh rows: r_l=0:256, r_r=256:512, z_l=512:768, z_r=768:1024
WRZH_PERM = [0, 4, 1, 5, 2, 6, 3, 7]

LAST_RESULT = {}


def _wavefront_order(nchunks, d=2):
    """Topological chunk order interleaving levels.

    Chunk (lv, ci)'s parents are (lv-1, 2ci) and (lv-1, 2ci+1) when level
    lv-1 has 2x the chunks; when levels shrink below NC the parent is the
    single previous-level chunk. Child front must come >= parent_pos + d
    (parent back emitted d steps after its front). Returns list of (lv, ci).
    """
    pos = {}
    order = []
    remaining = [(lv, ci) for lv in range(NLEVELS)
                 for ci in range(nchunks[lv])]

    def parents(lv, ci):
        if lv == 0:
            return []
        if nchunks[lv - 1] == 2 * nchunks[lv]:
            return [(lv - 1, 2 * ci), (lv - 1, 2 * ci + 1)]
        return [(lv - 1, pc) for pc in range(nchunks[lv - 1])]

    t = 0
    while remaining:
        ready = []
        for (lv, ci) in remaining:
            ps = parents(lv, ci)
            dd = d if lv > 1 else (d if lv == 1 else 0)
            if all(p in pos and pos[p] + (1 if lv == 1 else dd) <= t
                   for p in ps):
                ready.append((lv, ci))
        if ready:
            ch = max(ready, key=lambda c: (c[0], -c[1]))
            pos[ch] = t
            order.append(ch)
            remaining.remove(ch)
        else:
            order.append(None)   # spacing step (emit only a back)
        t += 1
    return order


def build_nc(fast_bias: bool):
    nc = bacc.Bacc("TRN2", target_bir_lowering=False, debug=False)

    d_x = nc.dram_tensor("xT", [2, 128, N0], F32R, kind="ExternalInput")
    d_wrzh = nc.dram_tensor("wrzh", [4, 128, 1024], F32R, kind="ExternalInput")
    d_wgrzx = nc.dram_tensor("wgrzx", [2, 128, 768], F32R, kind="ExternalInput")
    d_wgh = nc.dram_tensor("wgh", [2, 128, 256], F32R, kind="ExternalInput")
    d_bias = nc.dram_tensor("bias6", [6, 128, 1], F32, kind="ExternalInput")
    d_out = nc.dram_tensor("out", [2, 128, TOT], F32R, kind="ExternalOutput")

    x = d_x.ap()
    wrzh = d_wrzh.ap()
    wgrzx = d_wgrzx.ap()
    wgh = d_wgh.ap()
    bias6 = d_bias.ap()
    out = d_out.ap()

    mm = nc.tensor.matmul
    nchunks = [max(1, NCOLS[lv] // NC) for lv in range(NLEVELS)]

    with tile.TileContext(nc) as tc, ExitStack() as ctx:
        singles = ctx.enter_context(tc.tile_pool(name="singles", bufs=1))
        xpool = ctx.enter_context(tc.tile_pool(name="xpool", bufs=3))
        gates_pool = ctx.enter_context(tc.tile_pool(name="gates", bufs=2))
        gsb_pool = ctx.enter_context(tc.tile_pool(name="gsb", bufs=3))
        scratch = ctx.enter_context(tc.tile_pool(name="scratch", bufs=2))
        psum = ctx.enter_context(tc.tile_pool(name="psum", bufs=3, space="PSUM"))

        # --- load constants ---
        w_rzh = []
        for kc in range(4):
            t = singles.tile([128, 1024], F32R, tag=f"wrzh{kc}", name=f"wrzh{kc}")
            nc.sync.dma_start(out=t, in_=wrzh[kc])
            w_rzh.append(t)
        w_grzx = []
        for kc in range(2):
            t = singles.tile([128, 768], F32R, tag=f"wgrzx{kc}", name=f"wgrzx{kc}")
            nc.sync.dma_start(out=t, in_=wgrzx[kc])
            w_grzx.append(t)
        w_gh = []
        for kc in range(2):
            t = singles.tile([128, 256], F32R, tag=f"wgh{kc}", name=f"wgh{kc}")
            nc.sync.dma_start(out=t, in_=wgh[kc])
            w_gh.append(t)
        b_t = []
        for i in range(6):
            t = singles.tile([128, 1], F32, tag=f"b{i}", name=f"b{i}")
            nc.sync.dma_start(out=t, in_=bias6[i])
            b_t.append(t)
        # b_t: [0]=bg0 [1]=bg1 [2]=bA0 [3]=bA1 [4]=bB0 [5]=bB1

        h_t = [singles.tile([128, 2, NCOLS[l]], F32R, tag=f"h{l}",
                            name=f"h{l}")
               for l in range(NLEVELS)]

        state = {}   # (lv, ci) -> dict of tiles/views for the back phase

        def emit_leaf_front(ci):
            c0 = ci * NC
            x_c = []
            for kc in range(2):
                t = xpool.tile([128, NC], F32R, tag=f"x{kc}", name=f"x{kc}")
                nc.sync.dma_start(out=t, in_=x[kc, :, c0:c0 + NC])
                x_c.append(t)
            srzx = gates_pool.tile([128, 4, NC], F32, tag="gates", name="srzx")
            for q in range(2):
                ps = psum.tile([128, 2, NC], F32, tag="ps", name="ps_rzx")
                for mb in range(2):
                    col = 256 + (q * 2 + mb) * 128
                    for kc in range(2):
                        mm(ps[:, mb, :], w_grzx[kc][:, col:col + 128],
                           x_c[kc], start=(kc == 0), stop=(kc == 1))
                if fast_bias:
                    nc.scalar.activation(srzx[:, 2 * q:2 * q + 2, :], ps,
                                         SIGMOID, bias=1.0)
                else:
                    for mb in range(2):
                        nc.scalar.activation(
                            srzx[:, 2 * q + mb, :], ps[:, mb, :],
                            SIGMOID, bias=b_t[2 + 2 * q + mb])
            ps_gx = psum.tile([128, 2, NC], F32, tag="ps", name="ps_gx")
            for mb in range(2):
                for kc in range(2):
                    mm(ps_gx[:, mb, :], w_grzx[kc][:, 128 * mb:128 * mb + 128],
                       x_c[kc], start=(kc == 0), stop=(kc == 1))
            tg = gsb_pool.tile([128, 2, NC], F32, tag="gsb", name="tg")
            if fast_bias:
                nc.scalar.activation(tg, ps_gx, TANH, bias=0.0)
            else:
                for mb in range(2):
                    nc.scalar.activation(tg[:, mb, :], ps_gx[:, mb, :],
                                         TANH, bias=b_t[mb])
            zsum = scratch.tile([128, 2, NC], F32, tag="sa", name="zsum")
            nc.gpsimd.tensor_add(zsum, srzx[:, 0:2, :], srzx[:, 2:4, :])
            tt = scratch.tile([128, 2, NC], F32, tag="sb", name="tt")
            nc.vector.tensor_scalar(tt, zsum, -0.5, 1.0, MULT, ADD)
            nc.vector.tensor_mul(h_t[0][:, :, c0:c0 + NC], tt, tg)
            if ci == nchunks[0] - 1:
                for cb in range(2):
                    nc.sync.dma_start(out=out[cb, :, OFFS[0]:OFFS[1]],
                                      in_=h_t[0][:, cb, :])

        def emit_front(lv, ci):
            if lv == 0:
                emit_leaf_front(ci)
                return
            n = NCOLS[lv]
            hp = h_t[lv - 1]
            ncur = min(n, NC)
            c0 = ci * ncur
            ch = hp[:, :, 2 * c0:2 * c0 + 2 * ncur].rearrange(
                "p c (n two) -> p c two n", two=2)
            h_l = ch[:, :, 0, :]
            h_r = ch[:, :, 1, :]

            # gates blocks: [rl0, zl0, rl1, zl1, rr0, zr0, rr1, zr1]
            gt = gates_pool.tile([128, 8, ncur], F32, tag="gates", name="gt")
            if ncur <= 128 and fast_bias:
                ps = psum.tile([128, 8, ncur], F32, tag="ps8", name="ps_rz8", bufs=1)
                for q in range(4):
                    for mb in range(2):
                        col = (q * 2 + mb) * 128
                        for kc in range(4):
                            mm(ps[:, q * 2 + mb, :],
                               w_rzh[kc][:, col:col + 128],
                               ch[:, kc % 2, kc // 2, :],
                               start=(kc == 0), stop=(kc == 3))
                nc.scalar.activation(gt, ps, SIGMOID, bias=1.0)
            else:
                for q in range(4):
                    ps = psum.tile([128, 2, ncur], F32, tag="ps", name="ps_rz")
                    for mb in range(2):
                        col = (q * 2 + mb) * 128
                        for kc in range(4):
                            mm(ps[:, mb, :], w_rzh[kc][:, col:col + 128],
                               ch[:, kc % 2, kc // 2, :],
                               start=(kc == 0), stop=(kc == 3))
                    if fast_bias:
                        nc.scalar.activation(gt[:, 2 * q:2 * q + 2, :], ps,
                                             SIGMOID, bias=1.0)
                    else:
                        bi = (2, 3, 4, 5)[q]
                        nc.scalar.activation(gt[:, 2 * q:2 * q + 2, :], ps,
                                             SIGMOID, bias=b_t[bi])
            glv = gt[:, 0:4, :].rearrange("p (c two) n -> p two c n", two=2)
            grv = gt[:, 4:8, :].rearrange("p (c two) n -> p two c n", two=2)
            r_l, z_l = glv[:, 0], glv[:, 1]
            r_r, z_r = grv[:, 0], grv[:, 1]

            big = ncur >= 256
            e1 = nc.gpsimd if big else nc.vector
            # r-path: s = r_l*h_l + r_r*h_r   (feeds the g matmul)
            p = scratch.tile([128, 2, ncur], F32, tag="sa", name="p")
            q_ = scratch.tile([128, 2, ncur], F32, tag="sb", name="q")
            s = scratch.tile([128, 2, ncur], F32R, tag="sc", name="s")
            e1.tensor_mul(p, r_l, h_l)
            e1.tensor_mul(q_, r_r, h_r)
            nc.vector.tensor_add(s, p, q_)
            # z-path consumed here so the gates tile frees early:
            #   zh = z_l*h_l + z_r*h_r ; t = 1 - (z_l+z_r)/2
            a = scratch.tile([128, 2, ncur], F32, tag="sa", name="a")
            b = scratch.tile([128, 2, ncur], F32, tag="sb", name="b")
            zh = scratch.tile([128, 2, ncur], F32, tag="sd", name="zh")
            e1.tensor_mul(a, z_l, h_l)
            nc.vector.tensor_mul(b, z_r, h_r)
            nc.vector.tensor_add(zh, a, b)
            zs = scratch.tile([128, 2, ncur], F32, tag="sa", name="zs")
            tt = scratch.tile([128, 2, ncur], F32, tag="se", name="tt")
            (nc.gpsimd if big else nc.vector).tensor_add(zs, z_l, z_r)
            nc.vector.tensor_scalar(tt, zs, -0.5, 1.0, MULT, ADD)
            state[(lv, ci)] = dict(s=s, zh=zh, tt=tt, c0=c0, ncur=ncur)

        def emit_back(lv, ci):
            if lv == 0:
                return
            st = state.pop((lv, ci))
            s, zh, tt = st["s"], st["zh"], st["tt"]
            c0, ncur = st["c0"], st["ncur"]

            psg = psum.tile([128, 2, ncur], F32, tag="ps", name="ps_g")
            for mb in range(2):
                for kc in range(2):
                    mm(psg[:, mb, :], w_gh[kc][:, 128 * mb:128 * mb + 128],
                       s[:, kc, :], start=(kc == 0), stop=(kc == 1))
            g_sb = gsb_pool.tile([128, 2, ncur], F32, tag="gsb", name="g_sb")
            if fast_bias:
                nc.scalar.activation(g_sb, psg, TANH, bias=0.0)
            else:
                for mb in range(2):
                    nc.scalar.activation(g_sb[:, mb, :], psg[:, mb, :],
                                         TANH, bias=b_t[mb])
            # h = zh + t*g
            m = scratch.tile([128, 2, ncur], F32, tag="sb", name="m")
            nc.vector.tensor_mul(m, tt, g_sb)
            nc.vector.tensor_add(h_t[lv][:, :, c0:c0 + ncur], m, zh)
            if ci == nchunks[lv] - 1:
                for cb in range(2):
                    nc.sync.dma_start(out=out[cb, :, OFFS[lv]:OFFS[lv + 1]],
                                      in_=h_t[lv][:, cb, :])

        D = 2
        order = _wavefront_order(nchunks, D)

        def parent_list(lv, ci):
            if lv == 0:
                return []
            if nchunks[lv - 1] == 2 * nchunks[lv]:
                return [(lv - 1, 2 * ci), (lv - 1, 2 * ci + 1)]
            return [(lv - 1, pc) for pc in range(nchunks[lv - 1])]

        pending = []
        done = set()

        def pop_back():
            b = pending.pop(0)
            emit_back(*b)
            done.add(b)

        for ch in order:
            if ch is None:
                if pending:
                    pop_back()
                continue
            lv, ci = ch
            for par in parent_list(lv, ci):
                while par not in done:
                    pop_back()
            while len(pending) >= D:
                pop_back()
            emit_front(lv, ci)
            pending.append(ch)
        while pending:
            pop_back()

    nc.compile()
    return nc


def _prep_inputs(inputs, Wgrzx, bgrzx, Wrzh, Wgh):
    """Host-side shard + layout prep. Returns (in_maps, fast_bias)."""
    x = np.ascontiguousarray(inputs, dtype=np.float32)
    Wgrzx = np.asarray(Wgrzx, dtype=np.float32)
    bgrzx = np.asarray(bgrzx, dtype=np.float32)
    Wrzh = np.asarray(Wrzh, dtype=np.float32)
    Wgh = np.asarray(Wgh, dtype=np.float32)

    fast_bias = bool(
        np.all(bgrzx[:MEM] == 0.0) and np.all(bgrzx[MEM:] == 1.0))

    wgrzxT = np.ascontiguousarray(Wgrzx.T.reshape(2, 128, 768))
    wrzhT = Wrzh.T.reshape(512, 8, 128)
    wrzhT = np.ascontiguousarray(
        wrzhT[:, WRZH_PERM, :].reshape(512, 1024).reshape(4, 128, 1024))
    wghT = np.ascontiguousarray(Wgh.T.reshape(2, 128, 256))
    bias6 = np.ascontiguousarray(bgrzx.reshape(6, 128, 1))

    in_maps = []
    for c in range(NCORES):
        xc = x[c * BLOC:(c + 1) * BLOC].reshape(N0, IN_DIM)
        xT = np.ascontiguousarray(xc.T).reshape(2, 128, N0)
        in_maps.append({
            "xT": xT,
            "wrzh": wrzhT,
            "wgrzx": wgrzxT,
            "wgh": wghT,
            "bias6": bias6,
        })
    return in_maps, fast_bias


def _gather(results):
    """results: list of per-core {'out': [2,128,TOT]} -> [B, 2L-1, MEM]."""
    outs = []
    for c in range(len(results)):
        fm = results[c]["out"].reshape(MEM, TOT)
        levels = []
        for lv in range(NLEVELS):
            k = NCOLS[lv] // BLOC
            blk = fm[:, OFFS[lv]:OFFS[lv + 1]].reshape(MEM, BLOC, k)
            levels.append(blk.transpose(1, 2, 0))
        outs.append(np.concatenate(levels, axis=1))
    return np.ascontiguousarray(
        np.concatenate(outs, axis=0), dtype=np.float32)


def kernel(**inputs):
    in_maps, fast_bias = _prep_inputs(
        inputs["inputs"], inputs["Wgrzx"], inputs["bgrzx"],
        inputs["Wrzh"], inputs["Wgh"])
    nc = build_nc(fast_bias)
    trace = bool(int(os.environ.get("BTGRU_TRACE", "0")))
    res = run_bass_kernel_spmd(
        nc, in_maps, core_ids=list(range(NCORES)), trace=trace)
    LAST_RESULT.clear()
    LAST_RESULT["exec_time_ns"] = res.exec_time_ns
    LAST_RESULT["profile_json"] = res.profile_json
    return _gather(res.results)


# revision 12
# speedup vs baseline: 1.0977x; 1.0307x over previous
"""BinaryTreeGRU Trainium2 kernel.

Batch of B=64 complete binary trees (L=512 leaves, 1023 nodes each),
data-parallel over trees across 8 NeuronCores (8 trees/core).

Layout: feature-major ("folded"): every activation tensor lives in SBUF as
[128 partitions, 2 feature-blocks, n_nodes] (mem dim 256 = 2 blocks of 128).
Level l has N_l = 8 * 512 / 2^l node-columns per core, columns packed
tree-major so that node j's children sit at columns 2j and 2j+1 of the
previous level -- child selection is a stride-2 access pattern, no copies.

Per level:  rzh = Wrzh @ [h_l ; h_r]  (PE, fp32r, N=512 moving chunks)
            gates = sigmoid(rzh + b)  (ACT, fused across gate blocks)
            s = r_l*h_l + r_r*h_r     (GpSimd + DVE)
            g = tanh(Wgh @ s + bg)    (PE + ACT)
            h = z_l*(h_l - g/2) + z_r*(h_r - g/2) + g   (DVE/GpSimd)

Host side only reshapes/transposes numpy arrays for sharding and gathers
the result.
"""

import os
from contextlib import ExitStack

import numpy as np

import concourse.bass as bass
import concourse.mybir as mybir
import concourse.tile as tile
from concourse import bacc
from concourse.bass_utils import run_bass_kernel_spmd

F32 = mybir.dt.float32
F32R = mybir.dt.float32r
MULT = mybir.AluOpType.mult
ADD = mybir.AluOpType.add
SIGMOID = mybir.ActivationFunctionType.Sigmoid
TANH = mybir.ActivationFunctionType.Tanh
COPY = mybir.ActivationFunctionType.Copy

MEM = 256
IN_DIM = 256
B = 64
L = 512
NCORES = 8
BLOC = B // NCORES            # trees per core
N0 = BLOC * L                 # leaf columns per core = 4096
NLEVELS = 10                  # 4096,2048,...,8 columns
NCOLS = [N0 >> l for l in range(NLEVELS)]
TOT = sum(NCOLS)              # 8184
OFFS = np.cumsum([0] + NCOLS).tolist()
NC = 512                      # node-column chunk (fp32 matmul moving max)

# W-row 128-blocks of Wrzh arranged column order (quarters of the big
# gates tile):
#   q0=[r_l0, z_l0]  q1=[r_l1, z_l1]  q2=[r_r0, z_r0]  q3=[r_r1, z_r1]
# Wrzh rows: r_l=0:256, r_r=256:512, z_l=512:768, z_r=768:1024
WRZH_PERM = [0, 4, 1, 5, 2, 6, 3, 7]

LAST_RESULT = {}


def _wavefront_order(nchunks, d=2):
    """Topological chunk order interleaving levels.

    Chunk (lv, ci)'s parents are (lv-1, 2ci) and (lv-1, 2ci+1) when level
    lv-1 has 2x the chunks; when levels shrink below NC the parent is the
    single previous-level chunk. Child front must come >= parent_pos + d
    (parent back emitted d steps after its front). Returns list of (lv, ci).
    """
    pos = {}
    order = []
    remaining = [(lv, ci) for lv in range(NLEVELS)
                 for ci in range(nchunks[lv])]

    def parents(lv, ci):
        if lv == 0:
            return []
        if nchunks[lv - 1] == 2 * nchunks[lv]:
            return [(lv - 1, 2 * ci), (lv - 1, 2 * ci + 1)]
        return [(lv - 1, pc) for pc in range(nchunks[lv - 1])]

    t = 0
    while remaining:
        ready = []
        for (lv, ci) in remaining:
            ps = parents(lv, ci)
            dd = d if lv > 1 else (d if lv == 1 else 0)
            if all(p in pos and pos[p] + (2 if lv == 1 else dd + 1) <= t
                   for p in ps):
                ready.append((lv, ci))
        if ready:
            ch = max(ready, key=lambda c: (c[0], -c[1]))
            pos[ch] = t
            order.append(ch)
            remaining.remove(ch)
        else:
            order.append(None)   # spacing step (emit only a back)
        t += 1
    return order


def build_nc(fast_bias: bool):
    nc = bacc.Bacc("TRN2", target_bir_lowering=False, debug=False)

    d_x = nc.dram_tensor("xT", [2, 128, N0], F32R, kind="ExternalInput")
    d_wrzh = nc.dram_tensor("wrzh", [4, 128, 1024], F32R, kind="ExternalInput")
    d_wgrzx = nc.dram_tensor("wgrzx", [2, 128, 768], F32R, kind="ExternalInput")
    d_wgh = nc.dram_tensor("wgh", [2, 128, 256], F32R, kind="ExternalInput")
    d_bias = nc.dram_tensor("bias6", [6, 128, 1], F32, kind="ExternalInput")
    d_out = nc.dram_tensor("out", [2, 128, TOT], F32R, kind="ExternalOutput")

    x = d_x.ap()
    wrzh = d_wrzh.ap()
    wgrzx = d_wgrzx.ap()
    wgh = d_wgh.ap()
    bias6 = d_bias.ap()
    out = d_out.ap()

    mm = nc.tensor.matmul
    nchunks = [max(1, NCOLS[lv] // NC) for lv in range(NLEVELS)]

    with tile.TileContext(nc) as tc, ExitStack() as ctx:
        singles = ctx.enter_context(tc.tile_pool(name="singles", bufs=1))
        xpool = ctx.enter_context(tc.tile_pool(name="xpool", bufs=3))
        gates_pool = ctx.enter_context(tc.tile_pool(name="gates", bufs=2))
        gsb_pool = ctx.enter_context(tc.tile_pool(name="gsb", bufs=3))
        scratch = ctx.enter_context(tc.tile_pool(name="scratch", bufs=2))
        psum = ctx.enter_context(tc.tile_pool(name="psum", bufs=3, space="PSUM"))

        # --- load constants ---
        w_rzh = []
        for kc in range(4):
            t = singles.tile([128, 1024], F32R, tag=f"wrzh{kc}", name=f"wrzh{kc}")
            nc.sync.dma_start(out=t, in_=wrzh[kc])
            w_rzh.append(t)
        w_grzx = []
        for kc in range(2):
            t = singles.tile([128, 768], F32R, tag=f"wgrzx{kc}", name=f"wgrzx{kc}")
            nc.sync.dma_start(out=t, in_=wgrzx[kc])
            w_grzx.append(t)
        w_gh = []
        for kc in range(2):
            t = singles.tile([128, 256], F32R, tag=f"wgh{kc}", name=f"wgh{kc}")
            nc.sync.dma_start(out=t, in_=wgh[kc])
            w_gh.append(t)
        b_t = []
        for i in range(6):
            t = singles.tile([128, 1], F32, tag=f"b{i}", name=f"b{i}")
            nc.sync.dma_start(out=t, in_=bias6[i])
            b_t.append(t)
        # b_t: [0]=bg0 [1]=bg1 [2]=bA0 [3]=bA1 [4]=bB0 [5]=bB1

        h_t = [singles.tile([128, 2, NCOLS[l]], F32R, tag=f"h{l}",
                            name=f"h{l}")
               for l in range(NLEVELS)]

        state = {}   # (lv, ci) -> dict of tiles/views for the back phase

        def emit_leaf_front(ci):
            c0 = ci * NC
            x_c = []
            for kc in range(2):
                t = xpool.tile([128, NC], F32R, tag=f"x{kc}", name=f"x{kc}")
                nc.sync.dma_start(out=t, in_=x[kc, :, c0:c0 + NC])
                x_c.append(t)
            srzx = gates_pool.tile([128, 4, NC], F32, tag="gates", name="srzx")
            for q in range(2):
                ps = psum.tile([128, 2, NC], F32, tag="ps", name="ps_rzx")
                for mb in range(2):
                    col = 256 + (q * 2 + mb) * 128
                    for kc in range(2):
                        mm(ps[:, mb, :], w_grzx[kc][:, col:col + 128],
                           x_c[kc], start=(kc == 0), stop=(kc == 1))
                if fast_bias:
                    nc.scalar.activation(srzx[:, 2 * q:2 * q + 2, :], ps,
                                         SIGMOID, bias=1.0)
                else:
                    for mb in range(2):
                        nc.scalar.activation(
                            srzx[:, 2 * q + mb, :], ps[:, mb, :],
                            SIGMOID, bias=b_t[2 + 2 * q + mb])
            ps_gx = psum.tile([128, 2, NC], F32, tag="ps", name="ps_gx")
            for mb in range(2):
                for kc in range(2):
                    mm(ps_gx[:, mb, :], w_grzx[kc][:, 128 * mb:128 * mb + 128],
                       x_c[kc], start=(kc == 0), stop=(kc == 1))
            tg = gsb_pool.tile([128, 2, NC], F32, tag="gsb", name="tg")
            if fast_bias:
                nc.scalar.activation(tg, ps_gx, TANH, bias=0.0)
            else:
                for mb in range(2):
                    nc.scalar.activation(tg[:, mb, :], ps_gx[:, mb, :],
                                         TANH, bias=b_t[mb])
            zsum = scratch.tile([128, 2, NC], F32, tag="sa", name="zsum")
            nc.gpsimd.tensor_add(zsum, srzx[:, 0:2, :], srzx[:, 2:4, :])
            tt = scratch.tile([128, 2, NC], F32, tag="sb", name="tt")
            nc.scalar.activation(tt, zsum, COPY, bias=1.0, scale=-0.5)
            nc.vector.tensor_mul(h_t[0][:, :, c0:c0 + NC], tt, tg)
            if ci == nchunks[0] - 1:
                for cb in range(2):
                    nc.sync.dma_start(out=out[cb, :, OFFS[0]:OFFS[1]],
                                      in_=h_t[0][:, cb, :])

        def emit_front(lv, ci):
            if lv == 0:
                emit_leaf_front(ci)
                return
            n = NCOLS[lv]
            hp = h_t[lv - 1]
            ncur = min(n, NC)
            c0 = ci * ncur
            ch = hp[:, :, 2 * c0:2 * c0 + 2 * ncur].rearrange(
                "p c (n two) -> p c two n", two=2)
            h_l = ch[:, :, 0, :]
            h_r = ch[:, :, 1, :]

            # gates blocks: [rl0, zl0, rl1, zl1, rr0, zr0, rr1, zr1]
            gt = gates_pool.tile([128, 8, ncur], F32, tag="gates", name="gt")
            if ncur <= 128 and fast_bias:
                ps = psum.tile([128, 8, ncur], F32, tag="ps8", name="ps_rz8", bufs=1)
                for q in range(4):
                    for mb in range(2):
                        col = (q * 2 + mb) * 128
                        for kc in range(4):
                            mm(ps[:, q * 2 + mb, :],
                               w_rzh[kc][:, col:col + 128],
                               ch[:, kc % 2, kc // 2, :],
                               start=(kc == 0), stop=(kc == 3))
                nc.scalar.activation(gt, ps, SIGMOID, bias=1.0)
            else:
                for q in range(4):
                    ps = psum.tile([128, 2, ncur], F32, tag="ps", name="ps_rz")
                    for mb in range(2):
                        col = (q * 2 + mb) * 128
                        for kc in range(4):
                            mm(ps[:, mb, :], w_rzh[kc][:, col:col + 128],
                               ch[:, kc % 2, kc // 2, :],
                               start=(kc == 0), stop=(kc == 3))
                    if fast_bias:
                        nc.scalar.activation(gt[:, 2 * q:2 * q + 2, :], ps,
                                             SIGMOID, bias=1.0)
                    else:
                        bi = (2, 3, 4, 5)[q]
                        nc.scalar.activation(gt[:, 2 * q:2 * q + 2, :], ps,
                                             SIGMOID, bias=b_t[bi])
            glv = gt[:, 0:4, :].rearrange("p (c two) n -> p two c n", two=2)
            grv = gt[:, 4:8, :].rearrange("p (c two) n -> p two c n", two=2)
            r_l, z_l = glv[:, 0], glv[:, 1]
            r_r, z_r = grv[:, 0], grv[:, 1]

            big = ncur >= 256
            e1 = nc.gpsimd if big else nc.vector
            # r-path: s = r_l*h_l + r_r*h_r   (feeds the g matmul)
            p = scratch.tile([128, 2, ncur], F32, tag="sa", name="p")
            q_ = scratch.tile([128, 2, ncur], F32, tag="sb", name="q")
            s = scratch.tile([128, 2, ncur], F32R, tag="sc", name="s", bufs=3)
            e1.tensor_mul(p, r_l, h_l)
            e1.tensor_mul(q_, r_r, h_r)
            nc.vector.tensor_add(s, p, q_)
            # z-path consumed here so the gates tile frees early:
            #   zh = z_l*h_l + z_r*h_r ; t = 1 - (z_l+z_r)/2
            a = scratch.tile([128, 2, ncur], F32, tag="sa", name="a")
            b = scratch.tile([128, 2, ncur], F32, tag="sb", name="b")
            zh = scratch.tile([128, 2, ncur], F32, tag="sd", name="zh", bufs=3)
            nc.vector.tensor_mul(a, z_l, h_l)
            nc.vector.tensor_mul(b, z_r, h_r)
            nc.vector.tensor_add(zh, a, b)
            zs = scratch.tile([128, 2, ncur], F32, tag="sa", name="zs")
            tt = scratch.tile([128, 2, ncur], F32, tag="se", name="tt", bufs=3)
            (nc.gpsimd if big else nc.vector).tensor_add(zs, z_l, z_r)
            nc.scalar.activation(tt, zs, COPY, bias=1.0, scale=-0.5)
            state[(lv, ci)] = dict(s=s, zh=zh, tt=tt, c0=c0, ncur=ncur)

        def emit_back(lv, ci):
            if lv == 0:
                return
            st = state.pop((lv, ci))
            s, zh, tt = st["s"], st["zh"], st["tt"]
            c0, ncur = st["c0"], st["ncur"]

            psg = psum.tile([128, 2, ncur], F32, tag="ps", name="ps_g")
            for mb in range(2):
                for kc in range(2):
                    mm(psg[:, mb, :], w_gh[kc][:, 128 * mb:128 * mb + 128],
                       s[:, kc, :], start=(kc == 0), stop=(kc == 1))
            g_sb = gsb_pool.tile([128, 2, ncur], F32, tag="gsb", name="g_sb")
            if fast_bias:
                nc.scalar.activation(g_sb, psg, TANH, bias=0.0)
            else:
                for mb in range(2):
                    nc.scalar.activation(g_sb[:, mb, :], psg[:, mb, :],
                                         TANH, bias=b_t[mb])
            # h = zh + t*g
            m = scratch.tile([128, 2, ncur], F32, tag="sb", name="m")
            nc.vector.tensor_mul(m, tt, g_sb)
            nc.vector.tensor_add(h_t[lv][:, :, c0:c0 + ncur], m, zh)
            if ci == nchunks[lv] - 1:
                for cb in range(2):
                    nc.sync.dma_start(out=out[cb, :, OFFS[lv]:OFFS[lv + 1]],
                                      in_=h_t[lv][:, cb, :])

        D = 2
        order = _wavefront_order(nchunks, D)

        def parent_list(lv, ci):
            if lv == 0:
                return []
            if nchunks[lv - 1] == 2 * nchunks[lv]:
                return [(lv - 1, 2 * ci), (lv - 1, 2 * ci + 1)]
            return [(lv - 1, pc) for pc in range(nchunks[lv - 1])]

        pending = []
        done = set()

        def pop_back():
            b = pending.pop(0)
            emit_back(*b)
            done.add(b)

        for ch in order:
            if ch is None:
                if pending:
                    pop_back()
                continue
            lv, ci = ch
            for par in parent_list(lv, ci):
                while par not in done:
                    pop_back()
            emit_front(lv, ci)
            pending.append(ch)
            while len(pending) > D:
                pop_back()
        while pending:
            pop_back()

    nc.compile()
    return nc


def _prep_inputs(inputs, Wgrzx, bgrzx, Wrzh, Wgh):
    """Host-side shard + layout prep. Returns (in_maps, fast_bias)."""
    x = np.ascontiguousarray(inputs, dtype=np.float32)
    Wgrzx = np.asarray(Wgrzx, dtype=np.float32)
    bgrzx = np.asarray(bgrzx, dtype=np.float32)
    Wrzh = np.asarray(Wrzh, dtype=np.float32)
    Wgh = np.asarray(Wgh, dtype=np.float32)

    fast_bias = bool(
        np.all(bgrzx[:MEM] == 0.0) and np.all(bgrzx[MEM:] == 1.0))

    wgrzxT = np.ascontiguousarray(Wgrzx.T.reshape(2, 128, 768))
    wrzhT = Wrzh.T.reshape(512, 8, 128)
    wrzhT = np.ascontiguousarray(
        wrzhT[:, WRZH_PERM, :].reshape(512, 1024).reshape(4, 128, 1024))
    wghT = np.ascontiguousarray(Wgh.T.reshape(2, 128, 256))
    bias6 = np.ascontiguousarray(bgrzx.reshape(6, 128, 1))

    in_maps = []
    for c in range(NCORES):
        xc = x[c * BLOC:(c + 1) * BLOC].reshape(N0, IN_DIM)
        xT = np.ascontiguousarray(xc.T).reshape(2, 128, N0)
        in_maps.append({
            "xT": xT,
            "wrzh": wrzhT,
            "wgrzx": wgrzxT,
            "wgh": wghT,
            "bias6": bias6,
        })
    return in_maps, fast_bias


def _gather(results):
    """results: list of per-core {'out': [2,128,TOT]} -> [B, 2L-1, MEM]."""
    outs = []
    for c in range(len(results)):
        fm = results[c]["out"].reshape(MEM, TOT)
        levels = []
        for lv in range(NLEVELS):
            k = NCOLS[lv] // BLOC
            blk = fm[:, OFFS[lv]:OFFS[lv + 1]].reshape(MEM, BLOC, k)
            levels.append(blk.transpose(1, 2, 0))
        outs.append(np.concatenate(levels, axis=1))
    return np.ascontiguousarray(
        np.concatenate(outs, axis=0), dtype=np.float32)


def kernel(**inputs):
    in_maps, fast_bias = _prep_inputs(
        inputs["inputs"], inputs["Wgrzx"], inputs["bgrzx"],
        inputs["Wrzh"], inputs["Wgh"])
    nc = build_nc(fast_bias)
    trace = bool(int(os.environ.get("BTGRU_TRACE", "0")))
    res = run_bass_kernel_spmd(
        nc, in_maps, core_ids=list(range(NCORES)), trace=trace)
    LAST_RESULT.clear()
    LAST_RESULT["exec_time_ns"] = res.exec_time_ns
    LAST_RESULT["profile_json"] = res.profile_json
    return _gather(res.results)


# revision 19
# speedup vs baseline: 1.3508x; 1.2306x over previous
"""BinaryTreeGRU Trainium2 kernel.

Batch of B=64 complete binary trees (L=512 leaves, 1023 nodes each),
data-parallel over trees across 8 NeuronCores (8 trees/core).

Layout: feature-major ("folded"): every activation tensor lives in SBUF as
[128 partitions, 2 feature-blocks, n_nodes] (mem dim 256 = 2 blocks of 128).
Level l has N_l = 8 * 512 / 2^l node-columns per core, columns packed
tree-major so that node j's children sit at columns 2j and 2j+1 of the
previous level -- child selection is a stride-2 access pattern, no copies.

Per level:  rzh = Wrzh @ [h_l ; h_r]  (PE, fp32r, N=512 moving chunks)
            gates = sigmoid(rzh + b)  (ACT, fused across gate blocks)
            s = r_l*h_l + r_r*h_r     (GpSimd + DVE)
            g = tanh(Wgh @ s + bg)    (PE + ACT)
            h = z_l*(h_l - g/2) + z_r*(h_r - g/2) + g   (DVE/GpSimd)

Host side only reshapes/transposes numpy arrays for sharding and gathers
the result.
"""

import os
from contextlib import ExitStack

import numpy as np

import concourse.bass as bass
import concourse.mybir as mybir
import concourse.tile as tile
from concourse import bacc
from concourse.bass_utils import run_bass_kernel_spmd

F32 = mybir.dt.float32
F32R = mybir.dt.float32r
BF16 = mybir.dt.bfloat16
MULT = mybir.AluOpType.mult
ADD = mybir.AluOpType.add
SIGMOID = mybir.ActivationFunctionType.Sigmoid
TANH = mybir.ActivationFunctionType.Tanh
COPY = mybir.ActivationFunctionType.Copy

MEM = 256
IN_DIM = 256
B = 64
L = 512
NCORES = 8
BLOC = B // NCORES            # trees per core
N0 = BLOC * L                 # leaf columns per core = 4096
NLEVELS = 10                  # 4096,2048,...,8 columns
NCOLS = [N0 >> l for l in range(NLEVELS)]
TOT = sum(NCOLS)              # 8184
OFFS = np.cumsum([0] + NCOLS).tolist()
NC = 512                      # node-column chunk (fp32 matmul moving max)

# W-row 128-blocks of Wrzh arranged column order (quarters of the big
# gates tile):
#   q0=[r_l0, r_l1]  q1=[z_l0, z_l1]  q2=[r_r0, r_r1]  q3=[z_r0, z_r1]
# Wrzh rows: r_l=0:256, r_r=256:512, z_l=512:768, z_r=768:1024
WRZH_PERM = [0, 1, 4, 5, 2, 3, 6, 7]

LAST_RESULT = {}


def _wavefront_order(nchunks, d=2):
    """Topological chunk order interleaving levels.

    Chunk (lv, ci)'s parents are (lv-1, 2ci) and (lv-1, 2ci+1) when level
    lv-1 has 2x the chunks; when levels shrink below NC the parent is the
    single previous-level chunk. Child front must come >= parent_pos + d
    (parent back emitted d steps after its front). Returns list of (lv, ci).
    """
    pos = {}
    order = []
    remaining = [(lv, ci) for lv in range(NLEVELS)
                 for ci in range(nchunks[lv])]

    def parents(lv, ci):
        if lv == 0:
            return []
        if nchunks[lv - 1] == 2 * nchunks[lv]:
            return [(lv - 1, 2 * ci), (lv - 1, 2 * ci + 1)]
        return [(lv - 1, pc) for pc in range(nchunks[lv - 1])]

    t = 0
    while remaining:
        ready = []
        for (lv, ci) in remaining:
            ps = parents(lv, ci)
            dd = d if lv > 1 else (d if lv == 1 else 0)
            if all(p in pos and pos[p] + (2 if lv == 1 else dd + 1) <= t
                   for p in ps):
                ready.append((lv, ci))
        if ready:
            ch = max(ready, key=lambda c: (c[0], -c[1]))
            pos[ch] = t
            order.append(ch)
            remaining.remove(ch)
        else:
            order.append(None)   # spacing step (emit only a back)
        t += 1
    return order


def build_nc(fast_bias: bool):
    nc = bacc.Bacc("TRN2", target_bir_lowering=False, debug=False)

    d_x = nc.dram_tensor("xT", [2, 128, N0], F32R, kind="ExternalInput")
    d_wrzh = nc.dram_tensor("wrzh", [4, 128, 1024], F32R, kind="ExternalInput")
    d_wgrzx = nc.dram_tensor("wgrzx", [2, 128, 768], F32R, kind="ExternalInput")
    d_wgh = nc.dram_tensor("wgh", [2, 128, 256], F32R, kind="ExternalInput")
    d_bias = nc.dram_tensor("bias6", [6, 128, 1], F32, kind="ExternalInput")
    d_out = nc.dram_tensor("out", [2, 128, TOT], F32R, kind="ExternalOutput")

    x = d_x.ap()
    wrzh = d_wrzh.ap()
    wgrzx = d_wgrzx.ap()
    wgh = d_wgh.ap()
    bias6 = d_bias.ap()
    out = d_out.ap()

    mm = nc.tensor.matmul
    nchunks = [max(1, NCOLS[lv] // NC) for lv in range(NLEVELS)]

    with tile.TileContext(nc) as tc, ExitStack() as ctx:
        singles = ctx.enter_context(tc.tile_pool(name="singles", bufs=1))
        xpool = ctx.enter_context(tc.tile_pool(name="xpool", bufs=2))
        gates_pool = ctx.enter_context(tc.tile_pool(name="gates", bufs=2))
        gsb_pool = ctx.enter_context(tc.tile_pool(name="gsb", bufs=2))
        scratch = ctx.enter_context(tc.tile_pool(name="scratch", bufs=2))
        psum = ctx.enter_context(tc.tile_pool(name="psum", bufs=3, space="PSUM"))

        # --- load constants ---
        w_rzh = []
        for kc in range(4):
            t = singles.tile([128, 1024], F32R, tag=f"wrzh{kc}", name=f"wrzh{kc}")
            nc.sync.dma_start(out=t, in_=wrzh[kc])
            w_rzh.append(t)
        w_grzx = []
        for kc in range(2):
            t = singles.tile([128, 768], F32R, tag=f"wgrzx{kc}", name=f"wgrzx{kc}")
            nc.sync.dma_start(out=t, in_=wgrzx[kc])
            w_grzx.append(t)
        w_gh = []
        for kc in range(2):
            t = singles.tile([128, 256], F32R, tag=f"wgh{kc}", name=f"wgh{kc}")
            nc.sync.dma_start(out=t, in_=wgh[kc])
            w_gh.append(t)
        b_t = []
        for i in range(6):
            t = singles.tile([128, 1], F32, tag=f"b{i}", name=f"b{i}")
            nc.sync.dma_start(out=t, in_=bias6[i])
            b_t.append(t)
        # b_t: [0]=bg0 [1]=bg1 [2]=bA0 [3]=bA1 [4]=bB0 [5]=bB1

        h_t = [singles.tile([128, 2, 2, max(1, NCOLS[l] // 2)], F32R,
                            tag=f"h{l % 2}", name=f"h{l}", bufs=1)
               for l in range(NLEVELS)]
        # deinterleaved bf16 copy: h_bf[p, cb, lr, j] = h[p, cb, 2j+lr]
        h_bf = [singles.tile([128, 2, 2, NCOLS[l] // 2], BF16,
                             tag=f"hbf{l % 2}", name=f"hbf{l}", bufs=1)
                for l in range(NLEVELS - 1)]
        w_ghbf = []
        for kc in range(2):
            t = singles.tile([128, 256], BF16, tag=f"wghbf{kc}",
                             name=f"wghbf{kc}")
            nc.gpsimd.dma_start(out=t, in_=w_gh[kc])
            w_ghbf.append(t)

        def _strip1(v):
            return bass.AP(tensor=v.tensor, offset=v.offset,
                           ap=[d for d in v.ap if d[1] != 1])

        def emit_hbf(lv, c0, ncur):
            for cb in range(2):
                for par in range(2):
                    nc.gpsimd.dma_start(
                        out=_strip1(h_bf[lv][:, cb, par, c0:c0 + ncur]),
                        in_=_strip1(h_t[lv][:, cb, par, c0:c0 + ncur]))

        state = {}   # (lv, ci) -> dict of tiles/views for the back phase

        def h_scatter(lv, c0, ncur):
            """Column-ordered [128, 2, ncur] view of h_t[lv] (parity layout)
            covering columns c0..c0+ncur: dims (cb, j, par) with par innermost."""
            t = h_t[lv]
            half = max(1, NCOLS[lv] // 2)
            j0 = c0 // 2
            n2 = ncur // 2
            base = t[:, 0, 0, 0:1]
            return bass.AP(tensor=t.tensor, offset=t.offset + j0,
                           ap=[list(t.ap[0]), [2 * half, 2], [1, n2],
                               [half, 2]])

        def emit_leaf_front(ci):
            c0 = ci * NC
            x_c = []
            for kc in range(2):
                t = xpool.tile([128, NC], F32R, tag=f"x{kc}", name=f"x{kc}")
                nc.sync.dma_start(out=t, in_=x[kc, :, c0:c0 + NC])
                x_c.append(t)
            srzx = gates_pool.tile([128, 4, NC], F32, tag="gates", name="srzx")
            for q in range(2):
                ps = psum.tile([128, 2, NC], F32, tag="ps", name="ps_rzx")
                for mb in range(2):
                    col = 256 + (q * 2 + mb) * 128
                    for kc in range(2):
                        mm(ps[:, mb, :], w_grzx[kc][:, col:col + 128],
                           x_c[kc], start=(kc == 0), stop=(kc == 1))
                if fast_bias:
                    nc.scalar.activation(srzx[:, 2 * q:2 * q + 2, :], ps,
                                         SIGMOID, bias=1.0)
                else:
                    for mb in range(2):
                        nc.scalar.activation(
                            srzx[:, 2 * q + mb, :], ps[:, mb, :],
                            SIGMOID, bias=b_t[2 + 2 * q + mb])
            ps_gx = psum.tile([128, 2, NC], F32, tag="ps", name="ps_gx")
            for mb in range(2):
                for kc in range(2):
                    mm(ps_gx[:, mb, :], w_grzx[kc][:, 128 * mb:128 * mb + 128],
                       x_c[kc], start=(kc == 0), stop=(kc == 1))
            tg = gsb_pool.tile([128, 2, NC], F32, tag="gsb", name="tg")
            if fast_bias:
                nc.scalar.activation(tg, ps_gx, TANH, bias=0.0)
            else:
                for mb in range(2):
                    nc.scalar.activation(tg[:, mb, :], ps_gx[:, mb, :],
                                         TANH, bias=b_t[mb])
            zsum = scratch.tile([128, 2, NC], F32, tag="sa", name="zsum")
            nc.gpsimd.tensor_add(zsum, srzx[:, 0:2, :], srzx[:, 2:4, :])
            tt = scratch.tile([128, 2, NC], F32, tag="sb", name="tt")
            nc.scalar.activation(tt, zsum, COPY, bias=1.0, scale=-0.5)
            nc.vector.tensor_mul(h_scatter(0, c0, NC), tt, tg)
            emit_hbf(0, c0 // 2, NC // 2)
            if ci == nchunks[0] - 1:
                for cb in range(2):
                    nc.sync.dma_start(out=out[cb, :, OFFS[0]:OFFS[1]],
                                      in_=h_t[0][:, cb, :, :])

        def emit_front(lv, ci):
            if lv == 0:
                emit_leaf_front(ci)
                return
            n = NCOLS[lv]
            hp = h_t[lv - 1]
            ncur = min(n, NC)
            c0 = ci * ncur
            h_l = hp[:, :, 0, c0:c0 + ncur]
            h_r = hp[:, :, 1, c0:c0 + ncur]
            hb_l = h_bf[lv - 1][:, :, 0, c0:c0 + ncur]
            hb_r = h_bf[lv - 1][:, :, 1, c0:c0 + ncur]

            # gr blocks [rl0, rl1, rr0, rr1] (bf16);
            # gz blocks [zl0, zl1, zr0, zr1] (f32)
            gr = gates_pool.tile([128, 4, ncur], BF16, tag="gr", name="gr")
            gz = gates_pool.tile([128, 4, ncur], F32, tag="gz", name="gz")
            for q in range(4):
                dst = (gr, gz)[q % 2]
                half = (q // 2) * 2
                ps = psum.tile([128, 2, ncur], F32, tag="ps", name="ps_rz")
                for mb in range(2):
                    col = (q * 2 + mb) * 128
                    for kc in range(4):
                        mm(ps[:, mb, :], w_rzh[kc][:, col:col + 128],
                           hp[:, kc % 2, kc // 2, c0:c0 + ncur],
                           start=(kc == 0), stop=(kc == 3))
                if fast_bias:
                    nc.scalar.activation(dst[:, half:half + 2, :], ps,
                                         SIGMOID, bias=1.0)
                else:
                    bi = (2, 2, 4, 4)[q]
                    for mb in range(2):
                        nc.scalar.activation(dst[:, half + mb, :],
                                             ps[:, mb, :],
                                             SIGMOID, bias=b_t[bi + mb])
            r_l, r_r = gr[:, 0:2, :], gr[:, 2:4, :]
            z_l, z_r = gz[:, 0:2, :], gz[:, 2:4, :]

            big = ncur >= 256
            e1 = nc.gpsimd if big else nc.vector
            # r-path (bf16, 2x): s = r_l*h_l + r_r*h_r
            p = scratch.tile([128, 2, ncur], BF16, tag="sa", name="p")
            q_ = scratch.tile([128, 2, ncur], BF16, tag="sb", name="q")
            s = scratch.tile([128, 2, ncur], BF16, tag="sc", name="s", bufs=3)
            nc.vector.tensor_mul(p, r_l, hb_l)
            nc.vector.tensor_mul(q_, r_r, hb_r)
            nc.vector.tensor_add(s, p, q_)
            # z-path (f32): zh = z_l*h_l + z_r*h_r ; zs = z_l + z_r
            a = scratch.tile([128, 2, ncur], F32, tag="sa", name="a")
            b = scratch.tile([128, 2, ncur], F32, tag="sb", name="b")
            zh = scratch.tile([128, 2, ncur], F32, tag="sd", name="zh", bufs=2)
            e1.tensor_mul(a, z_l, h_l)
            e1.tensor_mul(b, z_r, h_r)
            nc.vector.tensor_add(zh, a, b)
            zs = scratch.tile([128, 2, ncur], F32, tag="se", name="zs", bufs=2)
            nc.vector.tensor_add(zs, z_l, z_r)
            state[(lv, ci)] = dict(s=s, zh=zh, zs=zs, c0=c0, ncur=ncur)

        def emit_back(lv, ci):
            if lv == 0:
                return
            st = state.pop((lv, ci))
            s, zh, zs = st["s"], st["zh"], st["zs"]
            c0, ncur = st["c0"], st["ncur"]

            psg = psum.tile([128, 2, ncur], F32, tag="ps", name="ps_g")
            for mb in range(2):
                for kc in range(2):
                    mm(psg[:, mb, :], w_ghbf[kc][:, 128 * mb:128 * mb + 128],
                       s[:, kc, :], start=(kc == 0), stop=(kc == 1))
            g_sb = gsb_pool.tile([128, 2, ncur], F32, tag="gsb", name="g_sb")
            if fast_bias:
                nc.scalar.activation(g_sb, psg, TANH, bias=0.0)
            else:
                for mb in range(2):
                    nc.scalar.activation(g_sb[:, mb, :], psg[:, mb, :],
                                         TANH, bias=b_t[mb])
            tt = scratch.tile([128, 2, ncur], F32, tag="sh", name="tt")
            nc.scalar.activation(tt, zs, COPY, bias=1.0, scale=-0.5)
            # h = zh + t*g   (t*g computed in-place into tt)
            nc.vector.tensor_mul(tt, tt, g_sb)
            nc.vector.tensor_add(h_scatter(lv, c0, ncur), tt, zh)
            if lv < NLEVELS - 1:
                emit_hbf(lv, c0 // 2, max(1, ncur // 2))
            if ci == nchunks[lv] - 1:
                for cb in range(2):
                    nc.sync.dma_start(out=out[cb, :, OFFS[lv]:OFFS[lv + 1]],
                                      in_=h_t[lv][:, cb, :])

        D = 2
        order = _wavefront_order(nchunks, D)

        def parent_list(lv, ci):
            if lv == 0:
                return []
            if nchunks[lv - 1] == 2 * nchunks[lv]:
                return [(lv - 1, 2 * ci), (lv - 1, 2 * ci + 1)]
            return [(lv - 1, pc) for pc in range(nchunks[lv - 1])]

        pending = []
        done = set()

        def pop_back():
            b = pending.pop(0)
            emit_back(*b)
            done.add(b)

        for ch in order:
            if ch is None:
                if pending:
                    pop_back()
                continue
            lv, ci = ch
            for par in parent_list(lv, ci):
                while par not in done:
                    pop_back()
            emit_front(lv, ci)
            pending.append(ch)
            while len(pending) > D:
                pop_back()
        while pending:
            pop_back()

    nc.compile()
    return nc


def _prep_inputs(inputs, Wgrzx, bgrzx, Wrzh, Wgh):
    """Host-side shard + layout prep. Returns (in_maps, fast_bias)."""
    x = np.ascontiguousarray(inputs, dtype=np.float32)
    Wgrzx = np.asarray(Wgrzx, dtype=np.float32)
    bgrzx = np.asarray(bgrzx, dtype=np.float32)
    Wrzh = np.asarray(Wrzh, dtype=np.float32)
    Wgh = np.asarray(Wgh, dtype=np.float32)

    fast_bias = bool(
        np.all(bgrzx[:MEM] == 0.0) and np.all(bgrzx[MEM:] == 1.0))

    wgrzxT = np.ascontiguousarray(Wgrzx.T.reshape(2, 128, 768))
    wrzhT = Wrzh.T.reshape(512, 8, 128)
    wrzhT = np.ascontiguousarray(
        wrzhT[:, WRZH_PERM, :].reshape(512, 1024).reshape(4, 128, 1024))
    wghT = np.ascontiguousarray(Wgh.T.reshape(2, 128, 256))
    bias6 = np.ascontiguousarray(bgrzx.reshape(6, 128, 1))

    in_maps = []
    for c in range(NCORES):
        xc = x[c * BLOC:(c + 1) * BLOC].reshape(N0, IN_DIM)
        xT = np.ascontiguousarray(xc.T).reshape(2, 128, N0)
        in_maps.append({
            "xT": xT,
            "wrzh": wrzhT,
            "wgrzx": wgrzxT,
            "wgh": wghT,
            "bias6": bias6,
        })
    return in_maps, fast_bias


def _gather(results):
    """results: list of per-core {'out': [2,128,TOT]} -> [B, 2L-1, MEM]."""
    outs = []
    for c in range(len(results)):
        fm = results[c]["out"].reshape(MEM, TOT)
        levels = []
        for lv in range(NLEVELS):
            n = NCOLS[lv]
            blk = fm[:, OFFS[lv]:OFFS[lv + 1]]
            nat = np.empty_like(blk)
            nat[:, 0::2] = blk[:, :n // 2]
            nat[:, 1::2] = blk[:, n // 2:]
            k = n // BLOC
            levels.append(nat.reshape(MEM, BLOC, k).transpose(1, 2, 0))
        outs.append(np.concatenate(levels, axis=1))
    return np.ascontiguousarray(
        np.concatenate(outs, axis=0), dtype=np.float32)


def kernel(**inputs):
    in_maps, fast_bias = _prep_inputs(
        inputs["inputs"], inputs["Wgrzx"], inputs["bgrzx"],
        inputs["Wrzh"], inputs["Wgh"])
    nc = build_nc(fast_bias)
    trace = bool(int(os.environ.get("BTGRU_TRACE", "0")))
    res = run_bass_kernel_spmd(
        nc, in_maps, core_ids=list(range(NCORES)), trace=trace)
    LAST_RESULT.clear()
    LAST_RESULT["exec_time_ns"] = res.exec_time_ns
    LAST_RESULT["profile_json"] = res.profile_json
    return _gather(res.results)
